# revision 26
# baseline (speedup 1.0000x reference)
"""Trainium2 Bass kernel for nn_DMP_Model (7-branch GCN+diff_proj message passing).

Strategy (branch/expert parallel, per the sharding hint):
  - 7 classes -> cores 0..6 (core 7 duplicates class 6; its output is ignored).
  - Per class, aggregation A@t is done with host-precomputed "rounds":
    nodes are sorted by scatter-degree so round k gathers the k-th incident
    edge's neighbor row for a compact rank-prefix, via dma_gather (SWDGE,
    4 queues, queue locked to each gather's sem lane post-compile) + DVE
    accumulate.  All per-node tensors live as fp16 row tables in DRAM;
    layout flips node<->feat use PE identity-matmul transposes (the xbar
    DMA-transpose path measured 4.4 GB/s on this HW — 70x below model).
  - diff_proj factorization: msg[r] = (t[r] * sum_col t[col]) @ dW + deg[r]*db
    (exact, since x[row] is constant over a row's edges).
  - GCN aggregation uses u = dinv*t gathers with post-scale by dinv[dst].
  - fp16 storage/matmul operands, fp32 PSUM/statistics.
"""
import sys
sys.path.insert(0, "/opt/trn_rl_repo")
import numpy as np

# ---------------- problem constants (hardcoded per contract) ----------------
N = 20000
D = 512
H = 256
NCLS = 7
L = 2
E = 131072
B = 8192
EPS = 1e-5
NP = 20096          # 157 * 128 padded node count
C157 = NP // 128
ZR = NP             # zero-row index in tables
TBLR = NP + 128     # table rows (zero region = rows NP..NP+127)
SEG_SLOTS = 24      # acc slots (x128 ranks) per segment
NQ = 4              # SWDGE queues (ucode max 4). Invariant: a DMA semaphore may only be
                    # updated from ONE queue. Tile assigns SWDGE completion
                    # sems from 8 global lanes round-robin in SCHEDULED order,
                    # so queue_num is rewritten post-compile to match each
                    # gather's own sem lane (see _fix_swdge_queues). Single-
                    # queue gathers measured 27.8 GB/s vs 467 GB/s multi-queue.
CORE_CLASS = [0, 1, 2, 3, 4, 5, 6, 6]
N_CORES = 8

_SEGS = []
_lo = 0
while _lo < C157:
    s = min(SEG_SLOTS, C157 - _lo)
    _SEGS.append((_lo, _lo + s))
    _lo += s


def _wrap_idx(flat):
    """int64 flat -> wrapped int16 [128, len/16] (idx i at [i%16, i//16], x8)."""
    assert len(flat) % 128 == 0
    w = flat.astype(np.int16).reshape(-1, 16).T
    return np.tile(w, (8, 1))


def _mm_windows(nslots, w=4):
    out = []
    c = 0
    while c < nslots:
        out.append((c, min(w, nslots - c)))
        c += w
    return out


# ---------------- host-side schedule (common across classes) ----------------

def _build_sched(cnts):
    """cnts[k] = common rank-count of round k. Returns (entries, total_idx).
    entry = (seg, k, slot0, mm, off). Segment-major, round-minor order."""
    entries = []
    off = 0
    for s, (lo, hi) in enumerate(_SEGS):
        lo_r, hi_r = lo * 128, hi * 128
        for k, cnt in enumerate(cnts):
            n = min(cnt - lo_r, hi_r - lo_r)
            if n <= 0:
                continue
            m = (n + 127) // 128
            sl = 0
            while sl < m:
                mm = min(8, m - sl)
                entries.append((s, k, sl, mm, off))
                off += mm * 128
                sl += mm
    return entries, off


def _edge_matrix(tgt, src_val, K):
    """M[node, k] = src_val of k-th edge with target tgt (ZR pad)."""
    order = np.argsort(tgt, kind="stable")
    t_sorted = tgt[order]
    grp_start = np.searchsorted(t_sorted, np.arange(N))
    k_within = np.arange(len(tgt)) - grp_start[t_sorted]
    M = np.full((N, K), ZR, np.int64)
    M[t_sorted, k_within] = src_val[order]
    return M


def _host_prep(inputs):
    """Per-class index/weight arrays + common schedules."""
    edges = np.asarray(inputs["edges"])
    x = np.asarray(inputs["x"], np.float32)

    # shared BN-fold statistics (depend only on x)
    xm = x.mean(0, dtype=np.float64).astype(np.float32)        # [D]
    Cxx = (x.T @ x) / np.float32(N)                            # [D, D]

    cls = []
    for c in range(NCLS):
        row = edges[c, 0].astype(np.int64)
        col = edges[c, 1].astype(np.int64)
        coldeg = np.bincount(col, minlength=N)
        rowdeg = np.bincount(row, minlength=N)
        deg = coldeg + 1
        dinv = 1.0 / np.sqrt(deg.astype(np.float64))
        piB = np.argsort(-rowdeg, kind="stable")        # storage / pass-B order
        sid_of = np.empty(N, np.int64)
        sid_of[piB] = np.arange(N)
        # pass-A order: coldeg-desc, ties by storage id -> the round-0
        # permutation gather reads ascending runs (DRAM locality)
        piA = np.lexsort((sid_of, -coldeg))
        rankA_of = np.empty(N, np.int64)
        rankA_of[piA] = np.arange(N)
        cls.append(dict(row=row, col=col, coldeg=coldeg, rowdeg=rowdeg,
                        dinv=dinv, piA=piA, piB=piB, sid_of=sid_of,
                        rankA_of=rankA_of))

    KA = max(int(c["coldeg"].max()) for c in cls)
    KB = max(int(c["rowdeg"].max()) for c in cls)
    cntA = [NP] + [max(int((c["coldeg"] >= k).sum()) for c in cls)
                   for k in range(1, KA + 1)]
    cntB = [max(int((c["rowdeg"] >= k).sum()) for c in cls)
            for k in range(1, KB + 1)]
    schedA, lenA = _build_sched(cntA)
    schedB, lenB = _build_sched(cntB)

    def fill_idx(sched, total, Mget):
        flat = np.full(total, ZR, np.int64)
        for (s, k, sl, mm, off) in sched:
            lo = _SEGS[s][0] * 128 + sl * 128
            n = mm * 128
            vals = Mget(k, lo, n)
            flat[off:off + n] = vals
        return flat

    per_core = []
    f16 = np.float16
    for core in range(N_CORES):
        c = CORE_CLASS[core]
        cc = cls[c]
        sid_of, piA, piB = cc["sid_of"], cc["piA"], cc["piB"]

        # pass A: round 0 = self (u[sid of node]), k>=1 = sid of row of (k-1)-th col-edge
        MA = _edge_matrix(cc["col"], sid_of[cc["row"]], max(KA, 1))

        def getA(k, lo, n, MA=MA, cc=cc):
            ranks = np.arange(lo, lo + n)
            valid = ranks < N
            out = np.full(n, ZR, np.int64)
            nodes = np.zeros(n, np.int64)
            nodes[valid] = cc["piA"][ranks[valid]]
            if k == 0:
                out[valid] = cc["sid_of"][nodes[valid]]
            else:
                v2 = valid & (cc["coldeg"][nodes] >= k)
                out[v2] = MA[nodes[v2], k - 1]
            return out

        MB = _edge_matrix(cc["row"], sid_of[cc["col"]], max(KB, 1))

        def getB(k, lo, n, MB=MB, cc=cc):
            ranks = np.arange(lo, lo + n)
            valid = ranks < N
            out = np.full(n, ZR, np.int64)
            nodes = np.zeros(n, np.int64)
            nodes[valid] = cc["piB"][ranks[valid]]
            v2 = valid & (cc["rowdeg"][nodes] >= k + 1)
            out[v2] = MB[nodes[v2], k]
            return out

        idxa = _wrap_idx(fill_idx(schedA, lenA, getA))
        idxb = _wrap_idx(fill_idx(schedB, lenB, getB))

        idxh = np.full(NP, ZR, np.int64)
        idxh[:N] = cc["rankA_of"][piB]
        idxh = _wrap_idx(idxh)

        node_id = np.asarray(inputs["node_id"]).astype(np.int64)
        idxp = _wrap_idx(np.concatenate([sid_of[node_id[0]], sid_of[node_id[1]]]))

        def arr157(v):  # [N] -> [128, 157] with rank=(slot*128+p) at [p, slot]
            a = np.zeros(NP, np.float32)
            a[:len(v)] = v
            return np.ascontiguousarray(a.reshape(C157, 128).T)

        dinvA = arr157(cc["dinv"][piA].astype(np.float32))
        dinvS = arr157(cc["dinv"][piB].astype(np.float32))
        degS = np.zeros((1, NP), f16)
        degS[0, :N] = cc["rowdeg"][piB].astype(f16)

        xT = np.zeros((128, 4, NP), f16)
        xp = x[piB].astype(f16)          # [N, 512]
        xT[:, :, :N] = xp.T.reshape(4, 128, N).transpose(1, 0, 2)

        def wtile(w, KH, MH):            # [K, M] -> [128, KH, MH, 128]
            K_, M_ = w.shape
            out = np.zeros((128, KH, MH, 128), f16)
            wp = np.zeros((KH * 128, MH * 128), np.float32)
            wp[:K_, :M_] = w
            out[:] = wp.reshape(KH, 128, MH, 128).transpose(1, 0, 2, 3).astype(f16)
            return out

        # ---- BN folded into lin0 (stats depend only on x, W0) ----
        W0 = np.asarray(inputs["lin0_W"][c], np.float32)
        b0 = np.asarray(inputs["lin0_b"][c], np.float32)
        g_ = np.asarray(inputs["bn_gamma"][c], np.float32)
        be = np.asarray(inputs["bn_beta"][c], np.float32)
        mu = xm @ W0 + b0
        var = np.einsum("if,ij,jf->f", W0, Cxx, W0,
                        optimize=True) - (xm @ W0) ** 2
        sc = g_ / np.sqrt(np.maximum(var, 0) + EPS)
        w0t = wtile(W0 * sc[None, :], 4, 2)
        b0f = ((b0 - mu) * sc + be).reshape(2, 128).T.copy()   # [128, 2] bias
        gwt = np.stack([wtile(np.asarray(inputs["gcn_W"][c, l], np.float32), 2, 2)
                        for l in range(L)], 1)          # [128, L, 2, 2, 128]
        dwt = np.stack([wtile(np.asarray(inputs["dp_W"][c, l], np.float32) * 8.0, 2, 2)
                        for l in range(L)], 1)
        dbt = np.zeros((128, L, 2, 128), f16)
        for l in range(L):
            dbt[0, l] = np.asarray(inputs["dp_b"][c, l], np.float32).reshape(2, 128).astype(f16)
        gbt = np.zeros((128, L, 2), np.float32)
        for l in range(L):
            gbt[:, l, :] = np.asarray(inputs["gcn_b"][c, l], np.float32).reshape(2, 128).T
        # fc1/fc2 outputs reach ~9e4 for some classes — past fp16 max. Store
        # f1 and f2 scaled by 1/8 and f3 by 1/4 (relu commutes with positive
        # scale); each scale is folded into the next layer's weights.
        f1w_ = np.asarray(inputs["fc1_W"][c], np.float32).copy()
        f1w_[512:768] *= 32.0            # p stores x1*x2/32
        f1w_ *= 0.125                    # f1 stored /8
        f1wt = wtile(f1w_, 6, 3)
        # fc2_W: x8 to undo f1 scale, /8 for its own output scale -> net 1.0
        f2wt = wtile(np.asarray(inputs["fc2_W"][c], np.float32), 3, 2)
        f3w = np.zeros((128, 2, 16), f16)
        f3_ = np.asarray(inputs["fc3_W"][c], np.float32) * 2.0  # x8 undo, /4 own
        f3p = np.zeros((256, 16), np.float32)
        f3p[:192] = f3_
        f3w[:] = f3p.reshape(2, 128, 16).transpose(1, 0, 2).astype(f16)
        clsw = (np.asarray(inputs["cls_W"], np.float32)[16 * c:16 * (c + 1), :]
                * 4.0).astype(f16)       # [16, 7], x4 undoes f3 scale
        f1b = np.asarray(inputs["fc1_b"][c], np.float32).reshape(3, 128).T * 0.125
        f2b = np.zeros((128, 2), np.float32)
        f2b[:64, 1] = np.asarray(inputs["fc2_b"][c], np.float32)[128:]
        f2b[:, 0] = np.asarray(inputs["fc2_b"][c], np.float32)[:128]
        f2b *= 0.125
        f3b = np.asarray(inputs["fc3_b"][c], np.float32).reshape(16, 1) * 0.25

        per_core.append(dict(
            ident=np.eye(128, dtype=f16),
            xT=xT, idxa=idxa, idxb=idxb, idxh=idxh, idxp=idxp,
            dinvA=dinvA, dinvS=dinvS, degS=degS,
            w0t=w0t, gwt=gwt, dwt=dwt, dbt=dbt, gbt=gbt, b0f=b0f,
            f1wt=f1wt, f2wt=f2wt, f3w=f3w, clsw=clsw,
            f1b=f1b, f2b=f2b, f3b=f3b,
        ))
    return per_core, schedA, lenA, schedB, lenB


# ---------------- device kernel ----------------

def _pack8(entries):
    """Group consecutive (k, sl, mm, off) entries with contiguous off ranges
    into gather groups of total mm <= 8."""
    groups = []
    cur, tot = [], 0
    for e in entries:
        contig = cur and e[3] == cur[-1][3] + cur[-1][2] * 128
        if cur and (tot + e[2] > 8 or not contig):
            groups.append(cur)
            cur, tot = [], 0
        cur.append(e)
        tot += e[2]
    if cur:
        groups.append(cur)
    return groups


_STAGES = ("lin0", "passA", "passB", "cls")   # build-time stage filter (perf bisection)


def _build_bass(schedA, lenA, schedB, lenB, reps=1):
    stages = set(_STAGES)
    import concourse.bacc as bacc
    import concourse.bass as bass
    import concourse.tile as tile
    from concourse import mybir
    from concourse import library_config

    f16 = mybir.dt.float16
    f32 = mybir.dt.float32
    f8 = mybir.dt.float8e4
    i16 = mybir.dt.int16
    AF = mybir.ActivationFunctionType

    nc = bacc.Bacc("TRN2", target_bir_lowering=False, debug=False,
                   num_devices=N_CORES, num_swdge_queues=NQ)

    def inp(name, shape, dt):
        return nc.dram_tensor(name, shape, dt, kind="ExternalInput")

    xT = inp("xT", [128, 4, NP], f16)
    ident = inp("ident", [128, 128], f16)
    idxa = inp("idxa", [128, lenA // 16], i16)
    idxb = inp("idxb", [128, lenB // 16], i16)
    idxh = inp("idxh", [128, NP // 16], i16)
    idxp = inp("idxp", [128, 2 * B // 16], i16)
    dinvA = inp("dinvA", [128, C157], f32)
    dinvS = inp("dinvS", [128, C157], f32)
    degS = inp("degS", [1, NP], f16)
    w0t = inp("w0t", [128, 4, 2, 128], f16)
    gwt = inp("gwt", [128, L, 2, 2, 128], f16)
    dwt = inp("dwt", [128, L, 2, 2, 128], f16)
    dbt = inp("dbt", [128, L, 2, 128], f16)
    gbt = inp("gbt", [128, L, 2], f32)
    f1wt = inp("f1wt", [128, 6, 3, 128], f16)
    f2wt = inp("f2wt", [128, 3, 2, 128], f16)
    f3w = inp("f3w", [128, 2, 16], f16)
    clsw = inp("clsw", [16, 7], f16)
    f1b = inp("f1b", [128, 3], f32)
    f2b = inp("f2b", [128, 2], f32)
    f3b = inp("f3b", [16, 1], f32)
    b0f = inp("b0f", [128, 2], f32)
    o7 = nc.dram_tensor("o7", [7, B], f32, kind="ExternalOutput")

    def tblr(name, dt=f16):
        return nc.dram_tensor(name, [TBLR, H], dt, kind="Internal")

    T = [tblr("T0"), tblr("T1"), tblr("T2")]
    U = [tblr("U0"), tblr("U1")]
    HT = tblr("HT")

    def rows(t):  # [TBLR, H] -> [128, 158, H]
        return t.ap().rearrange("(c p) f -> p c f", p=128)

    qctr = [0]

    def gq():
        # provisional build-order rotation; overwritten post-compile by
        # _fix_swdge_queues to match the scheduled sem-lane assignment
        q = qctr[0] % NQ
        qctr[0] += 1
        return q

    with tile.TileContext(nc) as tc:
        nc.gpsimd.load_library(library_config.mlp)
        from contextlib import ExitStack
        with ExitStack() as top:
            pers = top.enter_context(tc.tile_pool(name="pers", bufs=1))
            # ---- persistent loads ----
            idxa_t = pers.tile([128, lenA // 16], i16)
            nc.sync.dma_start(out=idxa_t[:], in_=idxa[:])
            idxb_t = pers.tile([128, lenB // 16], i16)
            nc.sync.dma_start(out=idxb_t[:], in_=idxb[:])
            idxh_t = pers.tile([128, NP // 16], i16)
            nc.sync.dma_start(out=idxh_t[:], in_=idxh[:])
            idxp_t = pers.tile([128, 2 * B // 16], i16)
            nc.sync.dma_start(out=idxp_t[:], in_=idxp[:])
            dinvA_t = pers.tile([128, C157], f32)
            nc.sync.dma_start(out=dinvA_t[:], in_=dinvA[:])
            dinvS_t = pers.tile([128, C157], f32)
            nc.sync.dma_start(out=dinvS_t[:], in_=dinvS[:])
            w0_t = pers.tile([128, 4, 2, 128], f16)
            nc.sync.dma_start(out=w0_t[:], in_=w0t[:])
            gw_t = pers.tile([128, L, 2, 2, 128], f16)
            nc.sync.dma_start(out=gw_t[:], in_=gwt[:])
            dw_t = pers.tile([128, L, 2, 2, 128], f16)
            nc.sync.dma_start(out=dw_t[:], in_=dwt[:])
            db_t = pers.tile([128, L, 2, 128], f16)
            nc.sync.dma_start(out=db_t[:], in_=dbt[:])
            gb_t = pers.tile([128, L, 2], f32)
            nc.sync.dma_start(out=gb_t[:], in_=gbt[:])
            b0f_t = pers.tile([128, 2], f32)
            nc.sync.dma_start(out=b0f_t[:], in_=b0f[:])

            # zero regions of tables
            zt = pers.tile([128, H], f16)
            nc.vector.memset(zt[:], 0.0)
            for t in (T[0], T[1], T[2], U[0], U[1], HT):
                nc.sync.dma_start(out=rows(t)[:, C157, :], in_=zt[:])
            id_t = pers.tile([128, 128], f16)
            nc.sync.dma_start(out=id_t[:], in_=ident[:])
            f1w_t = pers.tile([128, 6, 3, 128], f16)
            nc.sync.dma_start(out=f1w_t[:], in_=f1wt[:])
            f2w_t = pers.tile([128, 3, 2, 128], f16)
            nc.sync.dma_start(out=f2w_t[:], in_=f2wt[:])
            f3w_t = pers.tile([128, 2, 16], f16)
            nc.sync.dma_start(out=f3w_t[:], in_=f3w[:])
            clsw_t = pers.tile([16, 7], f16)
            nc.sync.dma_start(out=clsw_t[:], in_=clsw[:])
            f1b_t = pers.tile([128, 3], f32)
            nc.sync.dma_start(out=f1b_t[:], in_=f1b[:])
            f2b_t = pers.tile([128, 2], f32)
            nc.sync.dma_start(out=f2b_t[:], in_=f2b[:])
            f3b_t = pers.tile([16, 1], f32)
            nc.sync.dma_start(out=f3b_t[:], in_=f3b[:])

            def peT(out3, in2, nb, psp, tag="peT"):
                # per-128-block transpose via PE identity matmul; replaces
                # dma_start_transpose (xbar measured 360us per 3MB = 4.4GB/s,
                # ~70x slower than modeled). out3[p, b, r] = in2[r, b*128+p].
                # PSUM->SBUF copies alternate ACT/DVE to split the load.
                for b0 in range(0, nb, 4):
                    w = min(4, nb - b0)
                    pt = psp.tile([128, 512], f32, tag=tag)
                    for j in range(w):
                        nc.tensor.matmul(pt[:, j * 128:(j + 1) * 128],
                                         lhsT=in2[:, (b0 + j) * 128:(b0 + j + 1) * 128],
                                         rhs=id_t[:], start=True, stop=True)
                    # all copies on ACT: modeled busy ACT 0.90ms < Pool
                    # 1.05ms ceiling; shifting half to DVE would push DVE
                    # to ~1.3ms and raise the critical path
                    nc.scalar.activation(
                        out=out3[:, b0:b0 + w, :],
                        in_=pt[:, :w * 128].rearrange("p (c r) -> p c r", r=128),
                        func=AF.Copy)

            # The whole pipeline body repeats `reps` times (reps>1 builds a
            # timing variant: per-exec HW time = slope of wall vs reps,
            # cancelling the ~69ms axon per-dispatch overhead).
            for _rep in range(reps):
              if _rep:
                  tc.strict_bb_all_engine_barrier()
              # ================= lin0 (BN folded host-side) =================
              with tc.tile_pool(name="lin0x", bufs=3) as lpx, \
                   tc.tile_pool(name="lin0w", bufs=3) as lpw, \
                   tc.tile_pool(name="lin0s", bufs=3) as lps, \
                   tc.tile_pool(name="psA", bufs=4, space="PSUM") as psA:
                  # per 4-slot window: matmul -> relu+bias -> peT -> write T0/U0
                  wins = _mm_windows(C157, 4)
                  for (c0, w) in (wins if "lin0" in stages else []):
                      xs = lpx.tile([128, 4, 512], f16, tag="xs")
                      nc.sync.dma_start(out=xs[:, :, :w * 128],
                                        in_=xT[:, :, c0 * 128:(c0 + w) * 128])
                      t0w = lpw.tile([128, 4, 2, 128], f16, tag="t0w")
                      for mh in range(2):
                          pt = psA.tile([128, 512], f32)
                          for kh in range(4):
                              nc.tensor.matmul(pt[:, :w * 128], lhsT=w0_t[:, kh, mh, :],
                                               rhs=xs[:, kh, :w * 128],
                                               start=(kh == 0), stop=(kh == 3))
                          nc.scalar.activation(
                              out=t0w[:, :w, mh, :],
                              in_=pt[:, :w * 128].rearrange("p (c r) -> p c r", r=128),
                              func=AF.Relu, bias=b0f_t[:, mh:mh + 1])
                      tnp = lps.tile([128, 4, 2, 128], f16, tag="tnp")
                      peT(tnp[:, :w].rearrange("p c h f -> p (c h) f"),
                          t0w[:, :w].rearrange("p c h r -> p (c h r)"),
                          w * 2, psA)
                      nc.sync.dma_start(
                          out=rows(T[0])[:, c0:c0 + w, :],
                          in_=tnp[:, :w].rearrange("p c h f -> p c (h f)"))
                      nc.vector.tensor_mul(
                          tnp[:, :w], tnp[:, :w],
                          dinvS_t[:, c0:c0 + w, None, None].to_broadcast([128, w, 2, 128]))
                      nc.sync.dma_start(
                          out=rows(U[0])[:, c0:c0 + w, :],
                          in_=tnp[:, :w].rearrange("p c h f -> p c (h f)"))

              # ================= 2 comp-gcn layers =================
              for l in range(L):
                  with tc.tile_pool(name="acc", bufs=3) as accp, \
                       tc.tile_pool(name="stag", bufs=6) as stagp, \
                       tc.tile_pool(name="xTt", bufs=2) as xTp, \
                       tc.tile_pool(name="outT", bufs=2) as outTp, \
                       tc.tile_pool(name="npb", bufs=1) as npbp, \
                       tc.tile_pool(name="deg", bufs=1) as degp, \
                       tc.tile_pool(name="psL", bufs=4, space="PSUM") as psL:
                      # ---------- pass A: GCN aggregate + @gw + relu -> HT ----------
                      entA = {}
                      for e in schedA:
                          entA.setdefault(e[0], []).append(e)
                      for s, (lo, hi) in (enumerate(_SEGS) if "passA" in stages else []):
                          S = hi - lo
                          acc = accp.tile([128, SEG_SLOTS, H], f16, tag="acc")
                          rest = []
                          for (_s, k, sl, mm, off) in entA[s]:
                              if k == 0 and "gonly" not in stages:
                                  # round 0 covers every slot once: gather
                                  # straight into acc, no copy
                                  nc.gpsimd.dma_gather(
                                      acc[:, sl:sl + mm, :], U[l][:],
                                      idxa_t[:, off // 16:(off + mm * 128) // 16],
                                      mm * 128, mm * 128, H, queue_num=gq())
                              else:
                                  rest.append((k, sl, mm, off))
                          # pack consecutive tail-round entries (contiguous in
                          # the idx array) into shared gathers: fewer SWDGE
                          # instructions -> less Pool fixed desc-gen cost
                          for grp in _pack8(rest):
                              tot = sum(g[2] for g in grp)
                              off0 = grp[0][3]
                              stg = stagp.tile([128, 8, H], f16, tag="stg")
                              nc.gpsimd.dma_gather(
                                  stg[:, :tot, :], U[l][:],
                                  idxa_t[:, off0 // 16:(off0 + tot * 128) // 16],
                                  tot * 128, tot * 128, H, queue_num=gq())
                              if "gonly" in stages:
                                  continue
                              o = 0
                              for (k, sl, mm, off) in grp:
                                  assert off == off0 + o * 128
                                  nc.vector.tensor_add(
                                      acc[:, sl:sl + mm, :],
                                      acc[:, sl:sl + mm, :], stg[:, o:o + mm, :])
                                  o += mm
                          if "gonly" in stages or "noout" in stages:
                              continue
                          # post-scale by dinv[dst] (rank-A order)
                          nc.vector.tensor_mul(
                              acc[:, :S], acc[:, :S],
                              dinvA_t[:, lo:hi, None].to_broadcast([128, S, H]))
                          accT = xTp.tile([128, SEG_SLOTS, 2, 128], f16, tag="xT")
                          peT(accT[:, :S].rearrange("p c h r -> p (c h) r"),
                              acc[:, :S].rearrange("p c f -> p (c f)"),
                              S * 2, psL)
                          hT = outTp.tile([128, SEG_SLOTS, 2, 128], f16, tag="oT")
                          for (c0, w) in _mm_windows(S, 4):
                              for mh in range(2):
                                  pt = psL.tile([128, 512], f32)
                                  for kh in range(2):
                                      nc.tensor.matmul(
                                          pt[:, :w * 128], lhsT=gw_t[:, l, kh, mh, :],
                                          rhs=accT[:, c0:c0 + w, kh, :],
                                          start=(kh == 0), stop=(kh == 1))
                                  nc.scalar.activation(
                                      out=hT[:, c0:c0 + w, mh, :],
                                      in_=pt[:, :w * 128].rearrange("p (c r) -> p c r", r=128),
                                      func=AF.Relu, bias=gb_t[:, l, mh:mh + 1])
                          hnp = npbp.tile([128, SEG_SLOTS, 2, 128], f16, tag="hnp")
                          peT(hnp[:, :S].rearrange("p c h f -> p (c h) f"),
                              hT[:, :S].rearrange("p c h r -> p (c h r)"),
                              S * 2, psL)
                          nc.sync.dma_start(
                              out=rows(HT)[:, lo:hi, :],
                              in_=hnp[:, :S].rearrange("p c h f -> p c (h f)"))

                      # ---------- pass B: dp aggregate, msg, combine -> T/U[l+1] ----------
                      # Combine for segment s-1 is emitted AFTER segment s's
                      # edge gathers so the h-gather (which waits on the full
                      # HT table) never heads the in-order Pool queue while
                      # independent edge gathers are available.
                      entB = {}
                      for e in schedB:
                          entB.setdefault(e[0], []).append(e)

                      def _combineB(lo, hi, prodT):
                          S = hi - lo
                          # h gather (rankA -> storage order), fp8 source
                          hst = npbp.tile([128, SEG_SLOTS, H], f16, tag="hst")
                          for g0 in range(0, S, 8):
                              gm = min(8, S - g0)
                              off2 = (lo + g0) * 128
                              nc.gpsimd.dma_gather(
                                  hst[:, g0:g0 + gm, :], HT[:],
                                  idxh_t[:, off2 // 16:(off2 + gm * 128) // 16],
                                  gm * 128, gm * 128, H, queue_num=gq())
                          # msg^T = prod^T@(8dW) + deg*db (feat-major psum -> SBUF)
                          # deg rhs [128, S*128]: row0 = rowdeg, rest 0
                          dsg = degp.tile([128, SEG_SLOTS * 128], f16, tag="dsg")
                          nc.vector.memset(dsg[:, :S * 128], 0.0)
                          nc.sync.dma_start(out=dsg[0:1, :S * 128],
                                            in_=degS[:, lo * 128:hi * 128])
                          msgS = outTp.tile([128, SEG_SLOTS, 2, 128], f16, tag="oT")
                          for (c0, w) in _mm_windows(S, 4):
                              for mh in range(2):
                                  pt = psL.tile([128, 512], f32)
                                  for kh in range(2):
                                      nc.tensor.matmul(
                                          pt[:, :w * 128], lhsT=dw_t[:, l, kh, mh, :],
                                          rhs=prodT[:, c0:c0 + w, kh, :],
                                          start=(kh == 0), stop=False)
                                  nc.tensor.matmul(
                                      pt[:, :w * 128], lhsT=db_t[:, l, mh, :],
                                      rhs=dsg[:, c0 * 128:(c0 + w) * 128],
                                      start=False, stop=True)
                                  nc.scalar.activation(
                                      out=msgS[:, c0:c0 + w, mh, :],
                                      in_=pt[:, :w * 128].rearrange("p (c r) -> p c r", r=128),
                                      func=AF.Copy)
                          # transpose msg to node-major, add h from the psum
                          # directly (DVE reads PSUM), write T/U[l+1]
                          tnp = npbp.tile([128, SEG_SLOTS, 2, 128], f16, tag="hnp")
                          m2 = msgS[:, :S].rearrange("p c h r -> p (c h r)")
                          t2 = tnp[:, :S].rearrange("p c h f -> p (c h) f")
                          h2 = hst[:, :S].rearrange("p c (h f) -> p (c h) f", h=2)
                          for b0 in range(0, S * 2, 4):
                              wb = min(4, S * 2 - b0)
                              pt = psL.tile([128, 512], f32, tag="peT")
                              for j in range(wb):
                                  nc.tensor.matmul(pt[:, j * 128:(j + 1) * 128],
                                                   lhsT=m2[:, (b0 + j) * 128:(b0 + j + 1) * 128],
                                                   rhs=id_t[:], start=True, stop=True)
                              nc.vector.tensor_add(
                                  t2[:, b0:b0 + wb, :],
                                  pt[:, :wb * 128].rearrange("p (c r) -> p c r", r=128),
                                  h2[:, b0:b0 + wb, :])
                          nc.sync.dma_start(
                              out=rows(T[l + 1])[:, lo:hi, :],
                              in_=tnp[:, :S].rearrange("p c h f -> p c (h f)"))
                          if l + 1 < L:
                              nc.vector.tensor_mul(
                                  tnp[:, :S], tnp[:, :S],
                                  dinvS_t[:, lo:hi, None, None].to_broadcast([128, S, 2, 128]))
                              nc.sync.dma_start(
                                  out=rows(U[l + 1])[:, lo:hi, :],
                                  in_=tnp[:, :S].rearrange("p c h f -> p c (h f)"))

                      pendB = None
                      for s, (lo, hi) in (enumerate(_SEGS) if "passB" in stages else []):
                          S = hi - lo
                          acc2 = accp.tile([128, SEG_SLOTS, H], f16, tag="acc")
                          ent_list = entB.get(s, [])
                          m0 = max((e[2] + e[3] for e in ent_list if e[1] == 0),
                                   default=0)   # slots covered by round 0
                          if m0 < S:
                              nc.vector.memset(acc2[:, m0:S], 0.0)
                          restB = []
                          for (_s, k, sl, mm, off) in ent_list:
                              if k == 0:
                                  nc.gpsimd.dma_gather(
                                      acc2[:, sl:sl + mm, :], T[l][:],
                                      idxb_t[:, off // 16:(off + mm * 128) // 16],
                                      mm * 128, mm * 128, H, queue_num=gq())
                              else:
                                  restB.append((k, sl, mm, off))
                          for grp in _pack8(restB):
                              tot = sum(g[2] for g in grp)
                              off0 = grp[0][3]
                              stg = stagp.tile([128, 8, H], f16, tag="stg")
                              nc.gpsimd.dma_gather(
                                  stg[:, :tot, :], T[l][:],
                                  idxb_t[:, off0 // 16:(off0 + tot * 128) // 16],
                                  tot * 128, tot * 128, H, queue_num=gq())
                              o = 0
                              for (k, sl, mm, off) in grp:
                                  assert off == off0 + o * 128
                                  nc.vector.tensor_add(
                                      acc2[:, sl:sl + mm, :],
                                      acc2[:, sl:sl + mm, :], stg[:, o:o + mm, :])
                                  o += mm
                          # prod = t * acc2 (in storage order)
                          tseg = npbp.tile([128, SEG_SLOTS, H], f16, tag="tseg")
                          nc.sync.dma_start(out=tseg[:, :S], in_=rows(T[l])[:, lo:hi, :])
                          nc.vector.scalar_tensor_tensor(
                              out=acc2[:, :S], in0=acc2[:, :S], scalar=0.125,
                              in1=tseg[:, :S], op0=mybir.AluOpType.mult,
                              op1=mybir.AluOpType.mult)
                          prodT = xTp.tile([128, SEG_SLOTS, 2, 128], f16, tag="xT")
                          peT(prodT[:, :S].rearrange("p c h r -> p (c h) r"),
                              acc2[:, :S].rearrange("p c f -> p (c f)"),
                              S * 2, psL)
                          if pendB is not None:
                              _combineB(*pendB)
                          pendB = (lo, hi, prodT)
                      if pendB is not None:
                          _combineB(*pendB)

              # ================= pair classifier =================
              with tc.tile_pool(name="fc", bufs=1) as fp, \
                   tc.tile_pool(name="psF", bufs=4, space="PSUM") as psF:
                  PB = 2048          # pairs per piece
                  CB = PB // 128     # 16 chunks
                  for pp in (range(B // PB) if "cls" in stages else []):
                      x12 = fp.tile([128, 2, CB, H], f16, tag="x12")
                      for half in range(2):
                          for g0 in range(0, CB, 8):
                              gm = min(8, CB - g0)
                              off2 = half * B + pp * PB + g0 * 128
                              nc.gpsimd.dma_gather(
                                  x12[:, half, g0:g0 + gm, :], T[2][:],
                                  idxp_t[:, off2 // 16:(off2 + gm * 128) // 16],
                                  gm * 128, gm * 128, H, queue_num=gq())
                      p = fp.tile([128, CB, 3, H], f16, tag="p")
                      nc.vector.tensor_copy(out=p[:, :, 0, :], in_=x12[:, 0])
                      nc.vector.tensor_copy(out=p[:, :, 1, :], in_=x12[:, 1])
                      nc.vector.scalar_tensor_tensor(
                          out=p[:, :, 2, :], in0=x12[:, 0], scalar=0.03125,
                          in1=x12[:, 1], op0=mybir.AluOpType.mult,
                          op1=mybir.AluOpType.mult)
                      pT = fp.tile([128, CB, 6, 128], f16, tag="pT")
                      peT(pT[:].rearrange("p c h r -> p (c h) r"),
                          p[:].rearrange("p c t f -> p (c t f)"),
                          CB * 6, psF, tag="ps3")
                      f1T = fp.tile([128, CB, 3, 128], f16, tag="f1T")
                      for (c0, w) in _mm_windows(CB, 4):
                          for mh in range(3):
                              pt = psF.tile([128, 512], f32)
                              for kh in range(6):
                                  nc.tensor.matmul(pt[:, :w * 128], lhsT=f1w_t[:, kh, mh, :],
                                                   rhs=pT[:, c0:c0 + w, kh, :],
                                                   start=(kh == 0), stop=(kh == 5))
                              nc.scalar.activation(
                                  out=f1T[:, c0:c0 + w, mh, :],
                                  in_=pt[:, :w * 128].rearrange("p (c r) -> p c r", r=128),
                                  func=AF.Relu, bias=f1b_t[:, mh:mh + 1])
                      f2T = fp.tile([128, CB, 2, 128], f16, tag="f2T")
                      for (c0, w) in _mm_windows(CB, 4):
                          for mh in range(2):
                              pt = psF.tile([128, 512], f32)
                              for kh in range(3):
                                  nc.tensor.matmul(pt[:, :w * 128], lhsT=f2w_t[:, kh, mh, :],
                                                   rhs=f1T[:, c0:c0 + w, kh, :],
                                                   start=(kh == 0), stop=(kh == 2))
                              nc.scalar.activation(
                                  out=f2T[:, c0:c0 + w, mh, :],
                                  in_=pt[:, :w * 128].rearrange("p (c r) -> p c r", r=128),
                                  func=AF.Relu, bias=f2b_t[:, mh:mh + 1])
                      p3T = fp.tile([16, CB, 128], f16, tag="p3T")
                      for (c0, w) in _mm_windows(CB, 4):
                          pt = psF.tile([128, 512], f32, tag="ps3")
                          for kh in range(2):
                              nc.tensor.matmul(pt[:16, :w * 128], lhsT=f3w_t[:, kh, :],
                                               rhs=f2T[:, c0:c0 + w, kh, :],
                                               start=(kh == 0), stop=(kh == 1))
                          nc.scalar.activation(
                              out=p3T[:, c0:c0 + w, :],
                              in_=pt[:16, :w * 128].rearrange("p (c r) -> p c r", r=128),
                              func=AF.Relu, bias=f3b_t[:])
                      o7sb = fp.tile([7, CB, 128], f32, tag="o7sb")
                      for (c0, w) in _mm_windows(CB, 4):
                          pt = psF.tile([128, 512], f32, tag="ps3")
                          nc.tensor.matmul(pt[:7, :w * 128], lhsT=clsw_t[:],
                                           rhs=p3T[:, c0:c0 + w, :],
                                           start=True, stop=True)
                          nc.vector.tensor_copy(
                              out=o7sb[:, c0:c0 + w, :],
                              in_=pt[:7, :w * 128].rearrange("p (c r) -> p c r", r=128))
                      nc.sync.dma_start(
                          out=o7.ap().rearrange("q (c r) -> q c r", r=128)[:, pp * CB:(pp + 1) * CB, :],
                          in_=o7sb[:])

    nc.compile()
    _fix_swdge_queues(nc, mybir)
    return nc


def _fix_swdge_queues(nc, mybir):
    """Post-scheduling queue assignment. Tile rotates SWDGE completion sems
    over 8 global lanes (DMASW0..7) in scheduled instruction order; a sem may
    only ever be updated from one queue. Setting queue_num = lane of the
    instruction's own DMA sem satisfies that by construction and spreads
    gathers over all NQ queues."""
    import re
    n = 0
    for bb in nc.m.functions[0].blocks:
        for inst in bb.instructions:
            if isinstance(inst, mybir.InstDMAGatherAnt):
                si = inst.sync_info
                for u in (si.on_update if si else []):
                    m = re.match(r"DMASW(\d+)_", u.ant_name or "")
                    if m:
                        inst.queue_num = int(m.group(1)) % NQ
                        n += 1
    assert n > 0 or _STAGES != ("lin0", "passA", "passB", "cls"), \
        "no SWDGE gathers found to re-queue"


# ---------------- PJRT runner (compile once, run) ----------------

class _Runner:
    def __init__(self, nc, n_cores):
        import jax
        from jax.sharding import Mesh, PartitionSpec, NamedSharding
        from jax.experimental.shard_map import shard_map
        from concourse import mybir
        from concourse.bass2jax import (_bass_exec_p, install_neuronx_cc_hook,
                                        partition_id_tensor)
        install_neuronx_cc_hook()
        self.jax = jax
        self.n_cores = n_cores
        pname = nc.partition_id_tensor.name if nc.partition_id_tensor else None
        in_names, out_names, out_avals, zero_outs = [], [], [], []
        for alloc in nc.m.functions[0].allocations:
            if not isinstance(alloc, mybir.MemoryLocationSet):
                continue
            name = alloc.memorylocations[0].name
            if alloc.kind == "ExternalInput":
                if name != pname:
                    in_names.append(name)
            elif alloc.kind == "ExternalOutput":
                out_names.append(name)
                shape = tuple(alloc.tensor_shape)
                dtype = mybir.dt.np(alloc.dtype)
                out_avals.append(jax.core.ShapedArray(shape, dtype))
                zero_outs.append(np.zeros(shape, dtype))
        self.in_names, self.out_names = in_names, out_names
        self.out_avals, self.zero_outs = out_avals, zero_outs
        all_in = in_names + out_names + ([pname] if pname else [])

        def _body(*args):
            ops = list(args)
            if pname is not None:
                ops.append(partition_id_tensor())
            return tuple(_bass_exec_p.bind(
                *ops, out_avals=tuple(out_avals), in_names=tuple(all_in),
                out_names=tuple(out_names), lowering_input_output_aliases=(),
                sim_require_finite=False, sim_require_nnan=False, nc=nc))

        devices = jax.devices()[:n_cores]
        self.mesh = Mesh(np.asarray(devices), ("core",))
        specs = (PartitionSpec("core"),)
        self.fn = jax.jit(
            shard_map(_body, mesh=self.mesh,
                      in_specs=specs * (len(in_names) + len(out_names)),
                      out_specs=specs * len(out_names), check_rep=False),
            keep_unused=True)

    def prepare(self, in_maps):
        """Upload per-core inputs (and zeroed outputs) to the devices once."""
        jax = self.jax
        from jax.sharding import NamedSharding, PartitionSpec
        sh = NamedSharding(self.mesh, PartitionSpec("core"))
        concat = [np.concatenate([np.ascontiguousarray(in_maps[c][nm])
                                  for c in range(self.n_cores)], axis=0)
                  for nm in self.in_names]
        zeros = [np.zeros((self.n_cores * z.shape[0], *z.shape[1:]), z.dtype)
                 for z in self.zero_outs]
        dev = [jax.device_put(a, sh) for a in concat + zeros]
        jax.block_until_ready(dev)
        return dev

    def run_prepared(self, dev):
        """Execute on device-resident inputs; returns device output arrays."""
        outs = self.fn(*dev)
        self.jax.block_until_ready(outs)
        return outs

    def collect(self, outs):
        return [{nm: np.asarray(outs[i]).reshape(self.n_cores, *self.out_avals[i].shape)[c]
                 for i, nm in enumerate(self.out_names)}
                for c in range(self.n_cores)]

    def run(self, in_maps):
        return self.collect(self.run_prepared(self.prepare(in_maps)))


_CACHE = {}


def _get_runner(schedA, lenA, schedB, lenB, reps=1):
    key = (lenA, lenB, tuple(e[:4] for e in schedA[:50]), len(schedA),
           len(schedB), reps)
    if key not in _CACHE:
        nc = _build_bass(schedA, lenA, schedB, lenB, reps=reps)
        _CACHE[key] = _Runner(nc, N_CORES)
    return _CACHE[key]


def kernel(**inputs) -> np.ndarray:
    per_core, schedA, lenA, schedB, lenB = _host_prep(inputs)
    runner = _get_runner(schedA, lenA, schedB, lenB)
    res = runner.run(per_core)
    cls_b = np.asarray(inputs["cls_b"], np.float32)
    out = np.zeros((B, NCLS), np.float32)
    for core in range(NCLS):  # cores 0..6 hold classes 0..6
        out += res[core]["o7"].T
    out += cls_b[None, :]
    return out.astype(np.float32)

